# revision 47
# baseline (speedup 1.0000x reference)
"""Trainium2 Bass kernel: 3D-window sparse multi-head attention.

Full op: out = SDPA(hid@Wq, hid@Wk, hid@Wv; 3D local window mask) @ Wo + bo
Shapes: hid [1, 2048, 1024], 16 heads x 64, grid (8 frames, 16, 16), window (3, 5, 5).

Sharding: head-parallel. Each of the 8 cores computes 2 heads end-to-end
(QKV projection slices, windowed attention, Wo row-slice projection) and
writes a full-shape fp16 partial; the host sums the 8 partials and adds bo.

v2 over the fp16 baseline (61.1us -> 56.5us TimelineSim):
  * q/k/v projections run in fp8 (e4m3) DoubleRow perf mode: one DR matmul
    contracts TWO 128-row k-tiles at 0.5 cycles/col, so a D=1024 contraction
    takes 4 DRs instead of 8 fp16 matmuls.  Precision is recovered with a
    hi+lo split of BOTH operands (x = xh + xl/32, W = Wh/256 + Wl/256):
    three cross terms (xh*Wh, xl*Wp, xh*Wl) at product scale 256 accumulate
    in one PSUM group -> 12 DRs = 0.75x the fp16 PE cost, ~1.6e-3 rel err
    measured.  All scale factors are powers of 2 folded into host packing,
    the exp's constant scale (2^-19 = 1/(256*256*sqrt(hd))), and host-side
    Wo/256 (the v path carries a 256x scale through v1/oTn; fp16 range is
    fine: oTn sigma ~80, max ~1400).
  * software pipeline per iteration f: projection chunks land just before
    the first frame needing them, then scores(f) | o-half(f-2) | PV+norm
    (f-1) | o-half(f-2); the o-proj halves straddle the PV so pO copy
    latency never head-of-line blocks the in-order PE queue.
  * engine placement (hw rule: GPSIMD cannot touch PSUM): exp + q/k copies
    on ACT, masks + reciprocal + normalize + v copy on DVE, o-proj copies
    cycle DVE/ACT/DVE, broadcasts on GPSIMD.  Weight trios + m01 ride the
    SWDGE queue (descriptor gen on the idle Pool engine) because HWDGE
    descriptor generation is a single serial ~625ns/DMA resource.
  * PSUM banks (8): big512 pool (3 bufs) shared by q/k-proj and o-proj,
    paired two-head v tiles, psS x2, two-head pso [66, 2, 256] x2 (one
    [1,512] reciprocal per frame covers both heads' denominators).
  * tail: frame 7 normalizes per head and per seq-chunk half so the last
    o-proj starts early; its out-DMAs go per-half to overlap the
    issue+descriptor-gen+transfer+sem pipeline (~2.3us per DMA).
  * NOTE hw pitfalls (verified on device): GPSIMD+PSUM fails BIR
    verification; per-head K=64 o-proj accumulation (tail_split) dies at
    runtime; both are kept off.
"""

import numpy as np
import ml_dtypes

import concourse.bass as bass
import concourse.mybir as mybir
import concourse.tile as tile
from concourse import bacc
from concourse.bass import ds, ts
from concourse.bass_utils import run_bass_kernel_spmd

S, D, NH, HD = 2048, 1024, 16, 64
NCORES = 8
HPC = NH // NCORES          # heads per core = 2
F, GH, GW = 8, 16, 16       # frames, height, width (S = F*GH*GW)
WF, WH, WW = 3, 5, 5        # window sizes
T = GH * GW                 # tokens per frame = 256
P = 128
KC = D // P                 # 8 contraction chunks
SC = S // P                 # 16 seq chunks of 128
NQ = S // 512               # 4 free chunks of 512
LQ = 160                    # live q columns per kv chunk (h-band of the mask)
F16 = mybir.dt.float16
F32 = mybir.dt.float32
F8 = mybir.dt.float8e4
EXP = mybir.ActivationFunctionType.Exp
MUL = mybir.AluOpType.mult
DR = mybir.MatmulPerfMode.DoubleRow
NP8 = ml_dtypes.float8_e4m3fn
EXP_SCALE = float(2.0 ** -19)   # undo 256*256 packing and apply 1/sqrt(hd)

_nc_cache = {}

# engine-assignment / buffering knobs.
# NOTE: GPSIMD ("p") cannot touch PSUM on hardware — PSUM->SBUF copies and
# the normalize TT must stay on ACT ("a") / DVE ("d"); Pool is only usable
# for SBUF-only work (partition broadcast, the mask multiplies).
CFG = dict(
    et_bufs=6, pm_bufs=8, ob_bufs=8, pre_v=1, late_pat="a", tail_swdge=False, pvn_first=False, o6_pull=False, warmup=0, qk0_cs=False,
    eng_qk="a",        # psq/psk -> qT/kT copies: a=ACT, d=DVE
    eng_v="d",         # psv -> v1 copies
    eng_norm="d",      # normalize TT
    o_pat="dad",       # o-proj copy engines, cycling per copy
    mask_pat="d",      # mask-multiply engines, cycling per op
    dma_w_gp=True,     # wv3 on the gpsimd (SWDGE) queue
    dma_hid_vec=False, # stripe hid blocks across sync+scalar queues
    dma_out_sc=False,  # out DMAs on the scalar queue instead of sync
    tail_split=False,  # per-head K=64 o-proj: BROKEN on hw (runtime error)
)

_ENG = {"a": "scalar", "d": "vector", "p": "gpsimd"}


def _copy(nc, eng, dst, src):
    e = getattr(nc, _ENG[eng])
    if eng == "a":
        e.copy(dst, src)
    else:
        e.tensor_copy(dst, src)


def build_nc(debug=False, repeat=1, **cfg):
    c = dict(CFG, **cfg)
    key = (bool(debug), repeat, tuple(sorted(c.items())))
    if key in _nc_cache:
        return _nc_cache[key]
    nc = bacc.Bacc(None, target_bir_lowering=False, debug=False)

    xh = nc.dram_tensor("xh", [P, KC, S], F8, kind="ExternalInput")
    xl = nc.dram_tensor("xl", [P, KC, S], F8, kind="ExternalInput")
    # per-GEMM weight trios, term-major: [hi(256W), p(8W), lo(256*resid)];
    # wq is split so the hi term can land before the full trio streams in
    wqh = nc.dram_tensor("wqh", [P, KC, P], F8, kind="ExternalInput")
    wqpl = nc.dram_tensor("wqpl", [P, 2, KC, P], F8, kind="ExternalInput")
    wk3 = nc.dram_tensor("wk3", [P, 3, KC, P], F8, kind="ExternalInput")
    wv3 = nc.dram_tensor("wv3", [P, 3, KC, P], F8, kind="ExternalInput")
    wo = nc.dram_tensor("wo", [P, D], F16, kind="ExternalInput")
    m01 = nc.dram_tensor("m01", [P, 4, LQ], F16, kind="ExternalInput")
    out = nc.dram_tensor("out", [SC, P, D], F16, kind="ExternalOutput")
    dbg = {}
    if debug:
        dbg["qT"] = nc.dram_tensor("dbg_qt", [P, S], F16, kind="ExternalOutput")
        dbg["kT"] = nc.dram_tensor("dbg_kt", [P, S], F16, kind="ExternalOutput")
        dbg["v1"] = nc.dram_tensor("dbg_v1", [P, SC, HPC, 66], F16, kind="ExternalOutput")
        dbg["rp"] = nc.dram_tensor("dbg_rp", [1, 2 * S], F16, kind="ExternalOutput")
        dbg["oTn"] = nc.dram_tensor("dbg_otn", [P, S], F16, kind="ExternalOutput")

    with tile.TileContext(nc) as tc:
        with (
            tc.tile_pool(name="const", bufs=1) as cpool,
            tc.tile_pool(name="qk", bufs=1) as qkpool,
            tc.tile_pool(name="attn", bufs=4) as apool,
            tc.tile_pool(name="acc", bufs=1) as accpool,
            tc.tile_pool(name="ostage", bufs=4) as opool,
        ):
            # ---- constant loads. sync: wq (hi first) + xh block 0 pairs;
            # scalar: remaining xh-b0 pairs + all xl blocks; SWDGE: wk3, wv3,
            # m01 (Pool is idle early; each costs ~1us of descriptor gen).
            wqh_sb = cpool.tile([P, KC, P], F8, tag="wqh")
            wqpl_sb = cpool.tile([P, 2, KC, P], F8, tag="wqpl")
            wk3_sb = cpool.tile([P, 3, KC, P], F8, tag="wk3")
            wv3_sb = cpool.tile([P, 3, KC, P], F8, tag="wv3")
            xh_sb = cpool.tile([P, KC, S], F8, tag="xh")
            xl_sb = cpool.tile([P, KC, S], F8, tag="xl")
            m01_sb = cpool.tile([P, 4, LQ], F16, tag="m01")
            wo_sb = cpool.tile([P, D], F16, tag="wo")

            def xh_pair(eng, tp, nch=0):
                eng.dma_start(
                    xh_sb[:, 2 * tp : 2 * tp + 2, ts(nch, 512)],
                    xh[:, 2 * tp : 2 * tp + 2, ts(nch, 512)],
                )

            heng = nc.scalar if c["dma_hid_vec"] else nc.sync
            weng = nc.gpsimd if c["dma_w_gp"] else nc.sync
            nc.sync.dma_start(wqh_sb[:], wqh[:])
            if c["qk0_cs"]:
                # block 0 split by columns: frame 0's scores need only
                # q cols 0:256 and k cols 0:384, so cols 0:384 of xh/xl
                # unblock the whole first scores group
                nc.sync.dma_start(xh_sb[:, :, 0:384], xh[:, :, 0:384])
                weng.dma_start(wk3_sb[:], wk3[:])
                nc.sync.dma_start(wqpl_sb[:], wqpl[:])
                heng.dma_start(xl_sb[:, :, 0:384], xl[:, :, 0:384])
                nc.sync.dma_start(xh_sb[:, :, 384:512], xh[:, :, 384:512])
                heng.dma_start(xl_sb[:, :, 384:512], xl[:, :, 384:512])
            else:
                xh_pair(heng, 0)
                weng.dma_start(wk3_sb[:], wk3[:])
                nc.sync.dma_start(wqpl_sb[:], wqpl[:])
                xh_pair(heng, 2)
                xh_pair(nc.sync, 1)
                heng.dma_start(xl_sb[:, 0:4, ts(0, 512)], xl[:, 0:4, ts(0, 512)])
                xh_pair(nc.sync, 3)
                nc.sync.dma_start(xl_sb[:, 4:8, ts(0, 512)], xl[:, 4:8, ts(0, 512)])
            weng.dma_start(wv3_sb[:], wv3[:])
            weng.dma_start(m01_sb[:], m01[:])
            nc.sync.dma_start(xh_sb[:, :, ts(1, 512)], xh[:, :, ts(1, 512)])
            heng.dma_start(xl_sb[:, :, ts(1, 512)], xl[:, :, ts(1, 512)])
            nc.sync.dma_start(wo_sb[:], wo[:])
            for nch in range(2, NQ):
                nc.sync.dma_start(xh_sb[:, :, ts(nch, 512)], xh[:, :, ts(nch, 512)])
                heng.dma_start(xl_sb[:, :, ts(nch, 512)], xl[:, :, ts(nch, 512)])

            qT_sb = qkpool.tile([P, S], F16, tag="qT")
            kT_sb = qkpool.tile([P, S], F16, tag="kT")
            # sc-major v with an appended ones column for the denominator;
            # [P, sc, h, 66] lets one strided copy stage both heads of a
            # psv pair at once
            v1_sb = qkpool.tile([P, SC, HPC, 66], F16, tag="v1")
            oTn_sb = accpool.tile([P, S], F16, tag="oTn")
            # reciprocal denominators, frame-major [f*512 + h*256 + t]
            rp_sb = accpool.tile([1, HPC * S], F16, tag="rp")

            nc.vector.memset(v1_sb[:, :, 0, HD : HD + 2], 1.0)
            nc.vector.memset(v1_sb[:, :, 1, HD : HD + 2], 1.0)

            with (
                tc.tile_pool(name="big512", bufs=3, space="PSUM") as big512,
                tc.tile_pool(name="pv", bufs=1, space="PSUM") as pvp,
                tc.tile_pool(name="psS", bufs=2, space="PSUM") as pssp,
                tc.tile_pool(name="pso", bufs=2, space="PSUM") as psop,
                nc.allow_low_precision("softmax reciprocal in fp16"),
            ):
                o_ei = [0]  # cycling index into o_pat
                m_ei = [0]  # cycling index into mask_pat

                if c["warmup"]:
                    # dummy matmuls on a junk tile: the PE p-state ramps to
                    # full speed after ~3us of continuous execution, and the
                    # DMA-gated start would otherwise keep resetting it, so
                    # the first real DRs would run at half speed
                    junk = qkpool.tile([P, 512], F16, tag="junk")
                    nc.vector.memset(junk[:, 0:P], 0.0)
                    pw = pssp.tile([P, 3, LQ], F32, tag="psS", name="warm")
                    for i in range(c["warmup"]):
                        nc.tensor.matmul(
                            pw[:], junk[:, 0:P], junk[:, 0:480],
                            start=(i == 0), stop=(i == c["warmup"] - 1),
                        )

                def qw(term, t):
                    if term == 0:
                        return wqh_sb[:, 2 * t : 2 * t + 2, :]
                    return wqpl_sb[:, term - 1, 2 * t : 2 * t + 2, :]

                def kw(term, t):
                    return wk3_sb[:, term, 2 * t : 2 * t + 2, :]

                # term emission order: hi (xh), lo-W (xh), p (xl) — terms
                # needing xl go last so xl DMAs can trail xh
                TERMS = ((0, 0), (2, 0), (1, 1))

                def proj_drs(ps, wf, nch, terms=TERMS, n0=0, ntot=12):
                    """DoubleRow matmuls accumulating 256*(W^T x) for one
                    512-column chunk."""
                    n = n0
                    xs = (xh_sb, xl_sb)
                    for term, xi in terms:
                        for t in range(KC // 2):
                            nc.tensor.matmul(
                                ps[:], wf(term, t),
                                xs[xi][:, 2 * t : 2 * t + 2, ts(nch, 512)],
                                start=(n == 0), stop=(n == ntot - 1),
                                perf_mode=DR,
                            )
                            n += 1
                    return n

                def q_chunk(nch):
                    psq = big512.tile([P, 512], F32, tag="big", name="psq")
                    proj_drs(psq, qw, nch)
                    _copy(nc, c["eng_qk"], qT_sb[:, ts(nch, 512)], psq[:])

                def k_chunk(nch):
                    psk = big512.tile([P, 512], F32, tag="big", name="psk")
                    proj_drs(psk, kw, nch)
                    _copy(nc, c["eng_qk"], kT_sb[:, ts(nch, 512)], psk[:])

                def proj_drs_cols(ps, wf, c0, c1, xoff=0):
                    """12 DR accumulation over x columns [c0:c1)."""
                    n = 0
                    xs = (xh_sb, xl_sb)
                    for term, xi in TERMS:
                        for t in range(KC // 2):
                            nc.tensor.matmul(
                                ps[:, c0 - xoff : c1 - xoff], wf(term, t),
                                xs[xi][:, 2 * t : 2 * t + 2, c0:c1],
                                start=(n == 0), stop=(n == 11),
                                perf_mode=DR,
                            )
                            n += 1

                def qk_chunk(nch, interleave=False):
                    if interleave and c["qk0_cs"]:
                        # column-split: the A parts (q 0:256, k 0:384) are
                        # all frame 0's scores need; B parts follow once the
                        # 384:512 slices land
                        psq = big512.tile([P, 512], F32, tag="big", name="psq")
                        psk = big512.tile([P, 512], F32, tag="big", name="psk")
                        proj_drs_cols(psq, qw, 0, 256)
                        proj_drs_cols(psk, kw, 0, 384)
                        _copy(nc, c["eng_qk"], qT_sb[:, 0:256], psq[:, 0:256])
                        _copy(nc, c["eng_qk"], kT_sb[:, 0:384], psk[:, 0:384])
                        proj_drs_cols(psk, kw, 384, 512)
                        proj_drs_cols(psq, qw, 256, 512)
                        _copy(nc, c["eng_qk"], kT_sb[:, 384:512], psk[:, 384:512])
                        _copy(nc, c["eng_qk"], qT_sb[:, 256:512], psq[:, 256:512])
                    elif interleave:
                        # ordered by DMA arrival: wqh, wk3, wqpl, xl halves
                        psq = big512.tile([P, 512], F32, tag="big", name="psq")
                        psk = big512.tile([P, 512], F32, tag="big", name="psk")
                        for ti in range(3):
                            proj_drs(psq, qw, nch, terms=TERMS[ti : ti + 1],
                                     n0=4 * ti)
                            proj_drs(psk, kw, nch, terms=TERMS[ti : ti + 1],
                                     n0=4 * ti)
                        _copy(nc, c["eng_qk"], qT_sb[:, ts(nch, 512)], psq[:])
                        _copy(nc, c["eng_qk"], kT_sb[:, ts(nch, 512)], psk[:])
                    else:
                        q_chunk(nch)
                        k_chunk(nch)

                def v_pair(pair):
                    """v projection for seq chunks 2*pair, 2*pair+1."""
                    psv = pvp.tile([P, 2, HPC, HD], F32, tag="psv")
                    for j in range(2):
                        sc = 2 * pair + j
                        n = 0
                        for term, x_sb in ((0, xh_sb), (2, xh_sb), (1, xl_sb)):
                            for t in range(KC // 2):
                                nc.tensor.matmul(
                                    psv[:, j],
                                    x_sb[:, 2 * t : 2 * t + 2, ts(sc, P)],
                                    wv3_sb[:, term, 2 * t : 2 * t + 2, :],
                                    start=(n == 0), stop=(n == 11),
                                    perf_mode=DR,
                                )
                                n += 1
                    _copy(
                        nc, c["eng_v"],
                        v1_sb[:, 2 * pair : 2 * pair + 2, :, 0:HD],
                        psv[:],
                    )

                def o_half(f, j, pat=None, tail=False):
                    """output projection for seq chunk 2f+j."""
                    split = tail and c["tail_split"]
                    heads = (0, 1) if split else (None,)
                    sc = 2 * f + j
                    ob = opool.tile([P, D], F16, tag="ob", bufs=c["ob_bufs"])
                    oeng = nc.scalar if c["dma_out_sc"] else nc.sync
                    for n2 in range(2):
                        pO = big512.tile([P, 512], F32, tag="big", name="pO")
                        for i, h in enumerate(heads):
                            hr = ds(0, P) if h is None else ds(h * HD, HD)
                            nc.tensor.matmul(
                                pO[:], oTn_sb[hr, ts(sc, P)],
                                wo_sb[hr, ts(n2, 512)],
                                start=(i == 0), stop=(i == len(heads) - 1),
                            )
                        pp = pat or c["o_pat"]
                        e = pp[o_ei[0] % len(pp)]
                        o_ei[0] += 1
                        _copy(nc, e, ob[:, ts(n2, 512)], pO[:])
                        if tail:
                            # per-half DMAs; the last chunk's go through
                            # SWDGE so descriptor gen runs in parallel with
                            # the sync-queue HWDGE gens (shorter drain)
                            te = nc.gpsimd if (j == 1 and c["tail_swdge"]) else oeng
                            te.dma_start(out[sc, :, ts(n2, 512)], ob[:, ts(n2, 512)])
                    if not tail:
                        oeng.dma_start(out[sc, :, :], ob[:])

                def fgroups(f):
                    lo, hi = max(0, f - 1), min(F - 1, f + 1)
                    chunks = list(range(2 * lo, 2 * hi + 2))
                    return [chunks[i : i + 3] for i in range(0, len(chunks), 3)]

                pms = {}

                def scores_emk(f):
                    """scores + exp + mask for both heads of frame f.
                    The (h,w) window implies a kv-h band: an even kv chunk
                    (kv h 0..7) only reaches q columns 0:160, an odd chunk
                    (kv h 8..15) only 96:256. Tiles hold just those 160
                    live columns (m01 is host-packed the same way); PSUM's
                    per-element has_written bits make the partial-coverage
                    PV accumulation exact."""
                    for h in range(HPC):
                        hr = ds(h * HD, HD)
                        for b, g in enumerate(fgroups(f)):
                            n = len(g)
                            psS = pssp.tile([P, 3, LQ], F32, tag="psS")
                            for i, ck in enumerate(g):
                                nc.tensor.matmul(
                                    psS[:, i, :],
                                    kT_sb[hr, ds(ck * P, P)],
                                    qT_sb[hr, ds(f * T + 96 * (ck % 2), LQ)],
                                    start=True, stop=True,
                                )
                            et = apool.tile(
                                [P, 3, LQ], F16, tag="et", bufs=c["et_bufs"]
                            )
                            nc.scalar.activation(
                                et[:, 0:n, :], psS[:, 0:n, :], EXP, scale=EXP_SCALE
                            )
                            pm = apool.tile(
                                [P, 3, LQ], F16, tag="pm", bufs=c["pm_bufs"]
                            )
                            p0 = g[0] % 2
                            me = c["mask_pat"][m_ei[0] % len(c["mask_pat"])]
                            m_ei[0] += 1
                            meng = nc.gpsimd if me == "p" else nc.vector
                            meng.tensor_tensor(
                                pm[:, 0:n, :], et[:, 0:n, :],
                                m01_sb[:, p0 : p0 + n, :], MUL,
                            )
                            pms[(f, h, b)] = pm

                def pv_norm(f, split=False):
                    """PV for both heads into one two-head PSUM tile, then
                    one reciprocal over both denominator rows + per-head
                    GPSIMD broadcast + normalize multiply. With split=True
                    the normalize runs per seq-chunk half so the tail
                    o-projection can start off the first half early."""
                    fs = ds(f * T, T)
                    groups = fgroups(f)
                    pso = psop.tile([66, HPC, T], F32, tag="pso")
                    for h in range(HPC):
                        for b, g in enumerate(groups):
                            pm = pms.pop((f, h, b))
                            for i, ck in enumerate(g):
                                nc.tensor.matmul(
                                    pso[0 : HD + 1, h, ds(96 * (ck % 2), LQ)],
                                    v1_sb[:, ck, h, 0 : HD + 1],
                                    pm[:, i, :],
                                    start=(b == 0 and i == 0),
                                    stop=(b == len(groups) - 1 and i == len(g) - 1),
                                )
                    e = nc.vector if c["eng_norm"] == "d" else nc.gpsimd
                    if not split:
                        rps = ds(f * HPC * T, HPC * T)
                        nc.vector.reciprocal(rp_sb[0:1, rps], pso[HD : HD + 1, :, :])
                        for h in range(HPC):
                            rph = ds(f * HPC * T + h * T, T)
                            pbs = apool.tile([HD, T], F16, tag="pbs", bufs=2)
                            nc.gpsimd.partition_broadcast(pbs[:], rp_sb[0:1, rph])
                            e.tensor_tensor(
                                oTn_sb[ds(h * HD, HD), fs], pso[0:HD, h, :], pbs[:], MUL
                            )
                    else:
                        # tail variant: per-head reciprocal (h0's chain runs
                        # during h1's PV) and per-seq-chunk-half normalize so
                        # the final o-projection starts off half 0 early
                        pbss = {}
                        for h in range(HPC):
                            rph = ds(f * HPC * T + h * T, T)
                            nc.vector.reciprocal(
                                rp_sb[0:1, rph], pso[HD : HD + 1, h, :]
                            )
                            pbs = apool.tile([HD, T], F16, tag="pbs", bufs=2)
                            nc.gpsimd.partition_broadcast(pbs[:], rp_sb[0:1, rph])
                            pbss[h] = pbs
                        for half in range(2):
                            hs = ds(half * P, P)
                            for h in range(HPC):
                                e.tensor_tensor(
                                    oTn_sb[ds(h * HD, HD), ds(f * T + half * P, P)],
                                    pso[0:HD, h, hs], pbss[h][:, hs], MUL,
                                )

                # Software pipeline, per iteration f: projection chunks land
                # just before the first frame that needs them (vp_k by
                # iteration k, kT chunk n by the first frame whose kv band
                # reaches it, qT chunk n by frame 2n); scores(f) issue while
                # frame f-1's PV and frame f-2's output projection fill PE,
                # hiding the exp -> mask chain and the normalize chain
                # (recip -> broadcast -> multiply). Chunks are spread so
                # every iteration has PE filler.
                pres = {
                    1: {0: ["qk0", "vp0", "vp1"], 1: ["qk1", "vp2"],
                        2: ["vp3"], 3: ["qk2", "vp4"], 4: ["vp5"],
                        5: ["qk3", "vp6"], 6: ["vp7"], 7: []},
                    2: {0: ["qk0", "vp0"], 1: ["qk1", "vp1"],
                        2: ["vp2", "vp3"], 3: ["qk2", "vp4"],
                        4: ["vp5", "vp6"], 5: ["k3"], 6: ["q3", "vp7"],
                        7: []},
                    3: {0: ["qk0", "vp0"], 1: ["qk1", "vp1"],
                        2: ["vp2", "vp3"], 3: ["qk2", "vp4"],
                        4: ["vp5"], 5: ["k3", "vp6"], 6: ["q3", "vp7"],
                        7: []},
                    4: {0: ["qk0", "vp0", "vp1"], 1: ["qk1", "vp2"],
                        2: ["vp3", "vp4"], 3: ["qk2", "vp5"],
                        4: ["vp6"], 5: ["qk3", "vp7"], 6: [], 7: []},
                }
                pre = pres[c["pre_v"]]
                for rep in range(repeat):
                    for f in range(F):
                        for w in pre[f]:
                            if w.startswith("qk"):
                                nch = int(w[2:])
                                qk_chunk(nch, interleave=(nch == 0 and rep == 0))
                            elif w.startswith("q"):
                                q_chunk(int(w[1:]))
                            elif w.startswith("k"):
                                k_chunk(int(w[1:]))
                            else:
                                v_pair(int(w[2:]))
                        # o-proj halves straddle the PV so big512 pool
                        # rotation (pO copy latency) never head-of-line
                        # blocks the PE queue; late frames avoid DVE copies
                        # (DVE runs recip+normalize there)
                        pat = c["late_pat"] if f >= 6 else None
                        if c["pvn_first"]:
                            # pv_norm emitted before scores(f): GPSIMD's
                            # in-order queue then runs the norm broadcasts
                            # before frame f's (slow) Pool mask multiplies
                            if f >= 1:
                                pv_norm(f - 1)
                            if f >= 2:
                                o_half(f - 2, 0, pat=pat)
                            scores_emk(f)
                            if f >= 2:
                                o_half(f - 2, 1, pat=pat)
                        else:
                            scores_emk(f)
                            if f >= 2:
                                o_half(f - 2, 0, pat=pat)
                            if f >= 1:
                                pv_norm(f - 1)
                            if f >= 2:
                                o_half(f - 2, 1, pat=pat)
                        if f == F - 1:
                            o_half(F - 2, 0)
                            if c["o6_pull"]:
                                o_half(F - 2, 1)
                    pv_norm(F - 1, split=True)
                    if not c["o6_pull"]:
                        o_half(F - 2, 1)
                    o_half(F - 1, 0, pat="ad", tail=True)
                    o_half(F - 1, 1, pat="da", tail=True)

            if debug:
                nc.sync.dma_start(dbg["qT"][:], qT_sb[:])
                nc.sync.dma_start(dbg["kT"][:], kT_sb[:])
                nc.sync.dma_start(dbg["v1"][:], v1_sb[:])
                nc.sync.dma_start(dbg["rp"][:], rp_sb[:])
                nc.sync.dma_start(dbg["oTn"][:], oTn_sb[:])

    nc.compile()
    _nc_cache[key] = nc
    return nc


def _fp8_hilo(a32, lo_scale):
    """fp8 hi + fp8 of the scaled residual."""
    hi = a32.astype(NP8)
    lo = ((a32 - hi.astype(np.float32)) * lo_scale).astype(NP8)
    return hi, lo


def make_in_maps(hidden_states, Wq, Wk, Wv, Wo):
    """Host-side shard + repack of full inputs into per-core input maps."""
    hid = np.asarray(hidden_states, np.float32).reshape(S, D)
    # hidT packed [ki, ko, s] with d = ko*128 + ki
    hidT_pk = np.ascontiguousarray(hid.T.reshape(KC, P, S).transpose(1, 0, 2))
    xh8, xl8 = _fp8_hilo(hidT_pk, 32.0)

    Wq_ = np.asarray(Wq, np.float32)
    Wk_ = np.asarray(Wk, np.float32)
    Wv_ = np.asarray(Wv, np.float32)
    Wo_ = np.asarray(Wo, np.float32)

    def pack_w3(W, cidx):
        Wc = W[:, cidx * HPC * HD : (cidx + 1) * HPC * HD]  # [D, 128]
        wpk = np.ascontiguousarray(
            Wc.reshape(KC, P, HPC * HD).transpose(1, 0, 2)
        )
        hi = (wpk * 256.0).astype(NP8)
        lo = ((wpk * 256.0 - hi.astype(np.float32))).astype(NP8)
        wp = (wpk * 8.0).astype(NP8)
        return np.ascontiguousarray(np.stack([hi, wp, lo], axis=1))

    # (h, w) window mask, 0/1, [256, 256] (symmetric), packed to the live
    # window layout [p, j, c]: m01_pk[p, j, c] = W01[j*128 + p, 96*j + c]
    idx = np.arange(T)
    hh, ww = idx // GW, idx % GW
    m = (np.abs(hh[:, None] - hh[None, :]) <= WH // 2) & (
        np.abs(ww[:, None] - ww[None, :]) <= WW // 2
    )
    m01_pk = np.empty((P, 4, LQ), np.float16)
    for j in range(4):
        jp = j % 2
        m01_pk[:, j, :] = m[jp * P : (jp + 1) * P, 96 * jp : 96 * jp + LQ]

    in_maps = []
    for cidx in range(NCORES):
        wq3 = pack_w3(Wq_, cidx)
        in_maps.append(
            dict(
                xh=xh8,
                xl=xl8,
                wqh=np.ascontiguousarray(wq3[:, 0]),
                wqpl=np.ascontiguousarray(wq3[:, 1:]),
                wk3=pack_w3(Wk_, cidx),
                wv3=pack_w3(Wv_, cidx),
                wo=(
                    Wo_[cidx * HPC * HD : (cidx + 1) * HPC * HD, :] / 256.0
                ).astype(np.float16),
                m01=m01_pk,
            )
        )
    return in_maps


def kernel(
    hidden_states,
    Wq,
    Wk,
    Wv,
    Wo,
    bo,
    frames=F,
    height=GH,
    width=GW,
    wf=WF,
    wh=WH,
    ww=WW,
):
    assert (int(frames), int(height), int(width)) == (F, GH, GW)
    assert (int(wf), int(wh), int(ww)) == (WF, WH, WW)
    in_maps = make_in_maps(hidden_states, Wq, Wk, Wv, Wo)
    nc = build_nc(debug=False)
    res = run_bass_kernel_spmd(nc, in_maps, core_ids=list(range(NCORES)))
    acc = np.zeros((S, D), np.float32)
    for r in res.results:
        acc += r["out"].reshape(S, D)
    acc += np.asarray(bo, np.float32)[None, :]
    return acc.reshape(1, S, D)


# revision 51
# speedup vs baseline: 1.0070x; 1.0070x over previous
"""Trainium2 Bass kernel: 3D-window sparse multi-head attention.

Full op: out = SDPA(hid@Wq, hid@Wk, hid@Wv; 3D local window mask) @ Wo + bo
Shapes: hid [1, 2048, 1024], 16 heads x 64, grid (8 frames, 16, 16), window (3, 5, 5).

Sharding: head-parallel. Each of the 8 cores computes 2 heads end-to-end
(QKV projection slices, windowed attention, Wo row-slice projection) and
writes a full-shape fp16 partial; the host sums the 8 partials and adds bo.

v2 over the fp16 baseline (61.1us -> 56.5us TimelineSim):
  * q/k/v projections run in fp8 (e4m3) DoubleRow perf mode: one DR matmul
    contracts TWO 128-row k-tiles at 0.5 cycles/col, so a D=1024 contraction
    takes 4 DRs instead of 8 fp16 matmuls.  Precision is recovered with a
    hi+lo split of BOTH operands (x = xh + xl/32, W = Wh/256 + Wl/256):
    three cross terms (xh*Wh, xl*Wp, xh*Wl) at product scale 256 accumulate
    in one PSUM group -> 12 DRs = 0.75x the fp16 PE cost, ~1.6e-3 rel err
    measured.  All scale factors are powers of 2 folded into host packing,
    the exp's constant scale (2^-19 = 1/(256*256*sqrt(hd))), and host-side
    Wo/256 (the v path carries a 256x scale through v1/oTn; fp16 range is
    fine: oTn sigma ~80, max ~1400).
  * software pipeline per iteration f: projection chunks land just before
    the first frame needing them, then scores(f) | o-half(f-2) | PV+norm
    (f-1) | o-half(f-2); the o-proj halves straddle the PV so pO copy
    latency never head-of-line blocks the in-order PE queue.
  * engine placement (hw rule: GPSIMD cannot touch PSUM): exp + q/k copies
    on ACT, masks + reciprocal + normalize + v copy on DVE, o-proj copies
    cycle DVE/ACT/DVE, broadcasts on GPSIMD.  Weight trios + m01 ride the
    SWDGE queue (descriptor gen on the idle Pool engine) because HWDGE
    descriptor generation is a single serial ~625ns/DMA resource.
  * PSUM banks (8): big512 pool (3 bufs) shared by q/k-proj and o-proj,
    paired two-head v tiles, psS x2, two-head pso [66, 2, 256] x2 (one
    [1,512] reciprocal per frame covers both heads' denominators).
  * tail: frame 7 normalizes per head and per seq-chunk half so the last
    o-proj starts early; its out-DMAs go per-half to overlap the
    issue+descriptor-gen+transfer+sem pipeline (~2.3us per DMA).
  * NOTE hw pitfalls (verified on device): GPSIMD+PSUM fails BIR
    verification; per-head K=64 o-proj accumulation (tail_split) dies at
    runtime; both are kept off.
"""

import numpy as np
import ml_dtypes

import concourse.bass as bass
import concourse.mybir as mybir
import concourse.tile as tile
from concourse import bacc
from concourse.bass import ds, ts
from concourse.bass_utils import run_bass_kernel_spmd

S, D, NH, HD = 2048, 1024, 16, 64
NCORES = 8
HPC = NH // NCORES          # heads per core = 2
F, GH, GW = 8, 16, 16       # frames, height, width (S = F*GH*GW)
WF, WH, WW = 3, 5, 5        # window sizes
T = GH * GW                 # tokens per frame = 256
P = 128
KC = D // P                 # 8 contraction chunks
SC = S // P                 # 16 seq chunks of 128
NQ = S // 512               # 4 free chunks of 512
LQ = 160                    # live q columns per kv chunk (h-band of the mask)
F16 = mybir.dt.float16
F32 = mybir.dt.float32
F8 = mybir.dt.float8e4
EXP = mybir.ActivationFunctionType.Exp
MUL = mybir.AluOpType.mult
DR = mybir.MatmulPerfMode.DoubleRow
NP8 = ml_dtypes.float8_e4m3fn
EXP_SCALE = float(2.0 ** -19)   # undo 256*256 packing and apply 1/sqrt(hd)

_nc_cache = {}

# engine-assignment / buffering knobs.
# NOTE: GPSIMD ("p") cannot touch PSUM on hardware — PSUM->SBUF copies and
# the normalize TT must stay on ACT ("a") / DVE ("d"); Pool is only usable
# for SBUF-only work (partition broadcast, the mask multiplies).
CFG = dict(
    et_bufs=6, pm_bufs=8, ob_bufs=8, pre_v=1, late_pat="a", tail_swdge=False, pvn_first=False, o6_pull=False, warmup=0, qk0_cs=False, dma_coal=True, o_coal=False,
    eng_qk="a",        # psq/psk -> qT/kT copies: a=ACT, d=DVE
    eng_v="d",         # psv -> v1 copies
    eng_norm="d",      # normalize TT
    o_pat="adda",      # o-proj copy engines, cycling per copy
    mask_pat="d",      # mask-multiply engines, cycling per op
    dma_w_gp=True,     # wv3 on the gpsimd (SWDGE) queue
    dma_hid_vec=False, # stripe hid blocks across sync+scalar queues
    dma_out_sc=False,  # out DMAs on the scalar queue instead of sync
    tail_split=False,  # per-head K=64 o-proj: BROKEN on hw (runtime error)
)

_ENG = {"a": "scalar", "d": "vector", "p": "gpsimd"}


def _copy(nc, eng, dst, src):
    e = getattr(nc, _ENG[eng])
    if eng == "a":
        e.copy(dst, src)
    else:
        e.tensor_copy(dst, src)


def build_nc(debug=False, repeat=1, **cfg):
    c = dict(CFG, **cfg)
    key = (bool(debug), repeat, tuple(sorted(c.items())))
    if key in _nc_cache:
        return _nc_cache[key]
    nc = bacc.Bacc(None, target_bir_lowering=False, debug=False)

    xh = nc.dram_tensor("xh", [P, KC, S], F8, kind="ExternalInput")
    xl = nc.dram_tensor("xl", [P, KC, S], F8, kind="ExternalInput")
    # per-GEMM weight trios, term-major: [hi(256W), p(8W), lo(256*resid)];
    # wq is split so the hi term can land before the full trio streams in
    wqh = nc.dram_tensor("wqh", [P, KC, P], F8, kind="ExternalInput")
    wqpl = nc.dram_tensor("wqpl", [P, 2, KC, P], F8, kind="ExternalInput")
    wk3 = nc.dram_tensor("wk3", [P, 3, KC, P], F8, kind="ExternalInput")
    wv3 = nc.dram_tensor("wv3", [P, 3, KC, P], F8, kind="ExternalInput")
    wo = nc.dram_tensor("wo", [P, D], F16, kind="ExternalInput")
    m01 = nc.dram_tensor("m01", [P, 4, LQ], F16, kind="ExternalInput")
    out = nc.dram_tensor("out", [SC, P, D], F16, kind="ExternalOutput")
    dbg = {}
    if debug:
        dbg["qT"] = nc.dram_tensor("dbg_qt", [P, S], F16, kind="ExternalOutput")
        dbg["kT"] = nc.dram_tensor("dbg_kt", [P, S], F16, kind="ExternalOutput")
        dbg["v1"] = nc.dram_tensor("dbg_v1", [P, SC, HPC, 66], F16, kind="ExternalOutput")
        dbg["rp"] = nc.dram_tensor("dbg_rp", [1, 2 * S], F16, kind="ExternalOutput")
        dbg["oTn"] = nc.dram_tensor("dbg_otn", [P, S], F16, kind="ExternalOutput")

    with tile.TileContext(nc) as tc:
        with (
            tc.tile_pool(name="const", bufs=1) as cpool,
            tc.tile_pool(name="qk", bufs=1) as qkpool,
            tc.tile_pool(name="attn", bufs=4) as apool,
            tc.tile_pool(name="acc", bufs=1) as accpool,
            tc.tile_pool(name="ostage", bufs=4) as opool,
        ):
            # ---- constant loads. sync: wq (hi first) + xh block 0 pairs;
            # scalar: remaining xh-b0 pairs + all xl blocks; SWDGE: wk3, wv3,
            # m01 (Pool is idle early; each costs ~1us of descriptor gen).
            wqh_sb = cpool.tile([P, KC, P], F8, tag="wqh")
            wqpl_sb = cpool.tile([P, 2, KC, P], F8, tag="wqpl")
            wk3_sb = cpool.tile([P, 3, KC, P], F8, tag="wk3")
            wv3_sb = cpool.tile([P, 3, KC, P], F8, tag="wv3")
            xh_sb = cpool.tile([P, KC, S], F8, tag="xh")
            xl_sb = cpool.tile([P, KC, S], F8, tag="xl")
            m01_sb = cpool.tile([P, 4, LQ], F16, tag="m01")
            wo_sb = cpool.tile([P, D], F16, tag="wo")

            def xh_pair(eng, tp, nch=0):
                eng.dma_start(
                    xh_sb[:, 2 * tp : 2 * tp + 2, ts(nch, 512)],
                    xh[:, 2 * tp : 2 * tp + 2, ts(nch, 512)],
                )

            heng = nc.scalar if c["dma_hid_vec"] else nc.sync
            weng = nc.gpsimd if c["dma_w_gp"] else nc.sync
            nc.sync.dma_start(wqh_sb[:], wqh[:])
            if c["qk0_cs"]:
                # block 0 split by columns: frame 0's scores need only
                # q cols 0:256 and k cols 0:384, so cols 0:384 of xh/xl
                # unblock the whole first scores group
                nc.sync.dma_start(xh_sb[:, :, 0:384], xh[:, :, 0:384])
                weng.dma_start(wk3_sb[:], wk3[:])
                nc.sync.dma_start(wqpl_sb[:], wqpl[:])
                heng.dma_start(xl_sb[:, :, 0:384], xl[:, :, 0:384])
                nc.sync.dma_start(xh_sb[:, :, 384:512], xh[:, :, 384:512])
                heng.dma_start(xl_sb[:, :, 384:512], xl[:, :, 384:512])
            else:
                xh_pair(heng, 0)
                weng.dma_start(wk3_sb[:], wk3[:])
                nc.sync.dma_start(wqpl_sb[:], wqpl[:])
                xh_pair(heng, 2)
                xh_pair(nc.sync, 1)
                heng.dma_start(xl_sb[:, 0:4, ts(0, 512)], xl[:, 0:4, ts(0, 512)])
                xh_pair(nc.sync, 3)
                nc.sync.dma_start(xl_sb[:, 4:8, ts(0, 512)], xl[:, 4:8, ts(0, 512)])
            weng.dma_start(wv3_sb[:], wv3[:])
            weng.dma_start(m01_sb[:], m01[:])
            nc.sync.dma_start(xh_sb[:, :, ts(1, 512)], xh[:, :, ts(1, 512)])
            heng.dma_start(xl_sb[:, :, ts(1, 512)], xl[:, :, ts(1, 512)])
            nc.sync.dma_start(wo_sb[:], wo[:])
            if c["dma_coal"]:
                nc.sync.dma_start(xh_sb[:, :, 1024:2048], xh[:, :, 1024:2048])
                heng.dma_start(xl_sb[:, :, 1024:2048], xl[:, :, 1024:2048])
            else:
                for nch in range(2, NQ):
                    nc.sync.dma_start(xh_sb[:, :, ts(nch, 512)], xh[:, :, ts(nch, 512)])
                    heng.dma_start(xl_sb[:, :, ts(nch, 512)], xl[:, :, ts(nch, 512)])

            qT_sb = qkpool.tile([P, S], F16, tag="qT")
            kT_sb = qkpool.tile([P, S], F16, tag="kT")
            # sc-major v with an appended ones column for the denominator;
            # [P, sc, h, 66] lets one strided copy stage both heads of a
            # psv pair at once
            v1_sb = qkpool.tile([P, SC, HPC, 66], F16, tag="v1")
            oTn_sb = accpool.tile([P, S], F16, tag="oTn")
            # reciprocal denominators, frame-major [f*512 + h*256 + t]
            rp_sb = accpool.tile([1, HPC * S], F16, tag="rp")

            nc.vector.memset(v1_sb[:, :, 0, HD : HD + 2], 1.0)
            nc.vector.memset(v1_sb[:, :, 1, HD : HD + 2], 1.0)

            with (
                tc.tile_pool(name="big512", bufs=3, space="PSUM") as big512,
                tc.tile_pool(name="pv", bufs=1, space="PSUM") as pvp,
                tc.tile_pool(name="psS", bufs=2, space="PSUM") as pssp,
                tc.tile_pool(name="pso", bufs=2, space="PSUM") as psop,
                nc.allow_low_precision("softmax reciprocal in fp16"),
            ):
                o_ei = [0]  # cycling index into o_pat
                m_ei = [0]  # cycling index into mask_pat

                if c["warmup"]:
                    # dummy matmuls on a junk tile: the PE p-state ramps to
                    # full speed after ~3us of continuous execution, and the
                    # DMA-gated start would otherwise keep resetting it, so
                    # the first real DRs would run at half speed
                    junk = qkpool.tile([P, 512], F16, tag="junk")
                    nc.vector.memset(junk[:, 0:P], 0.0)
                    pw = pssp.tile([P, 3, LQ], F32, tag="psS", name="warm")
                    for i in range(c["warmup"]):
                        nc.tensor.matmul(
                            pw[:], junk[:, 0:P], junk[:, 0:480],
                            start=(i == 0), stop=(i == c["warmup"] - 1),
                        )

                def qw(term, t):
                    if term == 0:
                        return wqh_sb[:, 2 * t : 2 * t + 2, :]
                    return wqpl_sb[:, term - 1, 2 * t : 2 * t + 2, :]

                def kw(term, t):
                    return wk3_sb[:, term, 2 * t : 2 * t + 2, :]

                # term emission order: hi (xh), lo-W (xh), p (xl) — terms
                # needing xl go last so xl DMAs can trail xh
                TERMS = ((0, 0), (2, 0), (1, 1))

                def proj_drs(ps, wf, nch, terms=TERMS, n0=0, ntot=12):
                    """DoubleRow matmuls accumulating 256*(W^T x) for one
                    512-column chunk."""
                    n = n0
                    xs = (xh_sb, xl_sb)
                    for term, xi in terms:
                        for t in range(KC // 2):
                            nc.tensor.matmul(
                                ps[:], wf(term, t),
                                xs[xi][:, 2 * t : 2 * t + 2, ts(nch, 512)],
                                start=(n == 0), stop=(n == ntot - 1),
                                perf_mode=DR,
                            )
                            n += 1
                    return n

                def q_chunk(nch):
                    psq = big512.tile([P, 512], F32, tag="big", name="psq")
                    proj_drs(psq, qw, nch)
                    _copy(nc, c["eng_qk"], qT_sb[:, ts(nch, 512)], psq[:])

                def k_chunk(nch):
                    psk = big512.tile([P, 512], F32, tag="big", name="psk")
                    proj_drs(psk, kw, nch)
                    _copy(nc, c["eng_qk"], kT_sb[:, ts(nch, 512)], psk[:])

                def proj_drs_cols(ps, wf, c0, c1, xoff=0):
                    """12 DR accumulation over x columns [c0:c1)."""
                    n = 0
                    xs = (xh_sb, xl_sb)
                    for term, xi in TERMS:
                        for t in range(KC // 2):
                            nc.tensor.matmul(
                                ps[:, c0 - xoff : c1 - xoff], wf(term, t),
                                xs[xi][:, 2 * t : 2 * t + 2, c0:c1],
                                start=(n == 0), stop=(n == 11),
                                perf_mode=DR,
                            )
                            n += 1

                def qk_chunk(nch, interleave=False):
                    if interleave and c["qk0_cs"]:
                        # column-split: the A parts (q 0:256, k 0:384) are
                        # all frame 0's scores need; B parts follow once the
                        # 384:512 slices land
                        psq = big512.tile([P, 512], F32, tag="big", name="psq")
                        psk = big512.tile([P, 512], F32, tag="big", name="psk")
                        proj_drs_cols(psq, qw, 0, 256)
                        proj_drs_cols(psk, kw, 0, 384)
                        _copy(nc, c["eng_qk"], qT_sb[:, 0:256], psq[:, 0:256])
                        _copy(nc, c["eng_qk"], kT_sb[:, 0:384], psk[:, 0:384])
                        proj_drs_cols(psk, kw, 384, 512)
                        proj_drs_cols(psq, qw, 256, 512)
                        _copy(nc, c["eng_qk"], kT_sb[:, 384:512], psk[:, 384:512])
                        _copy(nc, c["eng_qk"], qT_sb[:, 256:512], psq[:, 256:512])
                    elif interleave:
                        # ordered by DMA arrival: wqh, wk3, wqpl, xl halves
                        psq = big512.tile([P, 512], F32, tag="big", name="psq")
                        psk = big512.tile([P, 512], F32, tag="big", name="psk")
                        for ti in range(3):
                            proj_drs(psq, qw, nch, terms=TERMS[ti : ti + 1],
                                     n0=4 * ti)
                            proj_drs(psk, kw, nch, terms=TERMS[ti : ti + 1],
                                     n0=4 * ti)
                        _copy(nc, c["eng_qk"], qT_sb[:, ts(nch, 512)], psq[:])
                        _copy(nc, c["eng_qk"], kT_sb[:, ts(nch, 512)], psk[:])
                    else:
                        q_chunk(nch)
                        k_chunk(nch)

                def v_pair(pair):
                    """v projection for seq chunks 2*pair, 2*pair+1."""
                    psv = pvp.tile([P, 2, HPC, HD], F32, tag="psv")
                    for j in range(2):
                        sc = 2 * pair + j
                        n = 0
                        for term, x_sb in ((0, xh_sb), (2, xh_sb), (1, xl_sb)):
                            for t in range(KC // 2):
                                nc.tensor.matmul(
                                    psv[:, j],
                                    x_sb[:, 2 * t : 2 * t + 2, ts(sc, P)],
                                    wv3_sb[:, term, 2 * t : 2 * t + 2, :],
                                    start=(n == 0), stop=(n == 11),
                                    perf_mode=DR,
                                )
                                n += 1
                    _copy(
                        nc, c["eng_v"],
                        v1_sb[:, 2 * pair : 2 * pair + 2, :, 0:HD],
                        psv[:],
                    )

                obf = {}

                def o_half(f, j, pat=None, tail=False):
                    """output projection for seq chunk 2f+j."""
                    split = tail and c["tail_split"]
                    heads = (0, 1) if split else (None,)
                    sc = 2 * f + j
                    if c["o_coal"] and not tail:
                        if j == 0:
                            obf[f] = opool.tile(
                                [P, 2, D], F16, tag="ob2",
                                bufs=c["ob_bufs"] // 2, name="ob2t",
                            )
                        ob = obf[f][:, j]
                    else:
                        ob = opool.tile([P, D], F16, tag="ob", bufs=c["ob_bufs"])
                    oeng = nc.scalar if c["dma_out_sc"] else nc.sync
                    for n2 in range(2):
                        pO = big512.tile([P, 512], F32, tag="big", name="pO")
                        for i, h in enumerate(heads):
                            hr = ds(0, P) if h is None else ds(h * HD, HD)
                            nc.tensor.matmul(
                                pO[:], oTn_sb[hr, ts(sc, P)],
                                wo_sb[hr, ts(n2, 512)],
                                start=(i == 0), stop=(i == len(heads) - 1),
                            )
                        pp = pat or c["o_pat"]
                        e = pp[o_ei[0] % len(pp)]
                        o_ei[0] += 1
                        _copy(nc, e, ob[:, ts(n2, 512)], pO[:])
                        if tail:
                            # per-half DMAs; the last chunk's go through
                            # SWDGE so descriptor gen runs in parallel with
                            # the sync-queue HWDGE gens (shorter drain)
                            te = nc.gpsimd if (j == 1 and c["tail_swdge"]) else oeng
                            te.dma_start(out[sc, :, ts(n2, 512)], ob[:, ts(n2, 512)])
                    if not tail:
                        if c["o_coal"]:
                            if j == 1:
                                oeng.dma_start(
                                    out[2 * f : 2 * f + 2, :, :], obf.pop(f)[:]
                                )
                        else:
                            oeng.dma_start(out[sc, :, :], ob[:])

                def fgroups(f):
                    lo, hi = max(0, f - 1), min(F - 1, f + 1)
                    chunks = list(range(2 * lo, 2 * hi + 2))
                    return [chunks[i : i + 3] for i in range(0, len(chunks), 3)]

                pms = {}

                def scores_emk(f):
                    """scores + exp + mask for both heads of frame f.
                    The (h,w) window implies a kv-h band: an even kv chunk
                    (kv h 0..7) only reaches q columns 0:160, an odd chunk
                    (kv h 8..15) only 96:256. Tiles hold just those 160
                    live columns (m01 is host-packed the same way); PSUM's
                    per-element has_written bits make the partial-coverage
                    PV accumulation exact."""
                    for h in range(HPC):
                        hr = ds(h * HD, HD)
                        for b, g in enumerate(fgroups(f)):
                            n = len(g)
                            psS = pssp.tile([P, 3, LQ], F32, tag="psS")
                            for i, ck in enumerate(g):
                                nc.tensor.matmul(
                                    psS[:, i, :],
                                    kT_sb[hr, ds(ck * P, P)],
                                    qT_sb[hr, ds(f * T + 96 * (ck % 2), LQ)],
                                    start=True, stop=True,
                                )
                            et = apool.tile(
                                [P, 3, LQ], F16, tag="et", bufs=c["et_bufs"]
                            )
                            nc.scalar.activation(
                                et[:, 0:n, :], psS[:, 0:n, :], EXP, scale=EXP_SCALE
                            )
                            pm = apool.tile(
                                [P, 3, LQ], F16, tag="pm", bufs=c["pm_bufs"]
                            )
                            p0 = g[0] % 2
                            me = c["mask_pat"][m_ei[0] % len(c["mask_pat"])]
                            m_ei[0] += 1
                            meng = nc.gpsimd if me == "p" else nc.vector
                            meng.tensor_tensor(
                                pm[:, 0:n, :], et[:, 0:n, :],
                                m01_sb[:, p0 : p0 + n, :], MUL,
                            )
                            pms[(f, h, b)] = pm

                def pv_norm(f, split=False):
                    """PV for both heads into one two-head PSUM tile, then
                    one reciprocal over both denominator rows + per-head
                    GPSIMD broadcast + normalize multiply. With split=True
                    the normalize runs per seq-chunk half so the tail
                    o-projection can start off the first half early."""
                    fs = ds(f * T, T)
                    groups = fgroups(f)
                    pso = psop.tile([66, HPC, T], F32, tag="pso")
                    for h in range(HPC):
                        for b, g in enumerate(groups):
                            pm = pms.pop((f, h, b))
                            for i, ck in enumerate(g):
                                nc.tensor.matmul(
                                    pso[0 : HD + 1, h, ds(96 * (ck % 2), LQ)],
                                    v1_sb[:, ck, h, 0 : HD + 1],
                                    pm[:, i, :],
                                    start=(b == 0 and i == 0),
                                    stop=(b == len(groups) - 1 and i == len(g) - 1),
                                )
                    e = nc.vector if c["eng_norm"] == "d" else nc.gpsimd
                    if not split:
                        rps = ds(f * HPC * T, HPC * T)
                        nc.vector.reciprocal(rp_sb[0:1, rps], pso[HD : HD + 1, :, :])
                        for h in range(HPC):
                            rph = ds(f * HPC * T + h * T, T)
                            pbs = apool.tile([HD, T], F16, tag="pbs", bufs=2)
                            nc.gpsimd.partition_broadcast(pbs[:], rp_sb[0:1, rph])
                            e.tensor_tensor(
                                oTn_sb[ds(h * HD, HD), fs], pso[0:HD, h, :], pbs[:], MUL
                            )
                    else:
                        # tail variant: per-head reciprocal (h0's chain runs
                        # during h1's PV) and per-seq-chunk-half normalize so
                        # the final o-projection starts off half 0 early
                        pbss = {}
                        for h in range(HPC):
                            rph = ds(f * HPC * T + h * T, T)
                            nc.vector.reciprocal(
                                rp_sb[0:1, rph], pso[HD : HD + 1, h, :]
                            )
                            pbs = apool.tile([HD, T], F16, tag="pbs", bufs=2)
                            nc.gpsimd.partition_broadcast(pbs[:], rp_sb[0:1, rph])
                            pbss[h] = pbs
                        for half in range(2):
                            hs = ds(half * P, P)
                            for h in range(HPC):
                                e.tensor_tensor(
                                    oTn_sb[ds(h * HD, HD), ds(f * T + half * P, P)],
                                    pso[0:HD, h, hs], pbss[h][:, hs], MUL,
                                )

                # Software pipeline, per iteration f: projection chunks land
                # just before the first frame that needs them (vp_k by
                # iteration k, kT chunk n by the first frame whose kv band
                # reaches it, qT chunk n by frame 2n); scores(f) issue while
                # frame f-1's PV and frame f-2's output projection fill PE,
                # hiding the exp -> mask chain and the normalize chain
                # (recip -> broadcast -> multiply). Chunks are spread so
                # every iteration has PE filler.
                pres = {
                    1: {0: ["qk0", "vp0", "vp1"], 1: ["qk1", "vp2"],
                        2: ["vp3"], 3: ["qk2", "vp4"], 4: ["vp5"],
                        5: ["qk3", "vp6"], 6: ["vp7"], 7: []},
                    2: {0: ["qk0", "vp0"], 1: ["qk1", "vp1"],
                        2: ["vp2", "vp3"], 3: ["qk2", "vp4"],
                        4: ["vp5", "vp6"], 5: ["k3"], 6: ["q3", "vp7"],
                        7: []},
                    3: {0: ["qk0", "vp0"], 1: ["qk1", "vp1"],
                        2: ["vp2", "vp3"], 3: ["qk2", "vp4"],
                        4: ["vp5"], 5: ["k3", "vp6"], 6: ["q3", "vp7"],
                        7: []},
                    4: {0: ["qk0", "vp0", "vp1"], 1: ["qk1", "vp2"],
                        2: ["vp3", "vp4"], 3: ["qk2", "vp5"],
                        4: ["vp6"], 5: ["qk3", "vp7"], 6: [], 7: []},
                }
                pre = pres[c["pre_v"]]
                for rep in range(repeat):
                    for f in range(F):
                        for w in pre[f]:
                            if w.startswith("qk"):
                                nch = int(w[2:])
                                qk_chunk(nch, interleave=(nch == 0 and rep == 0))
                            elif w.startswith("q"):
                                q_chunk(int(w[1:]))
                            elif w.startswith("k"):
                                k_chunk(int(w[1:]))
                            else:
                                v_pair(int(w[2:]))
                        # o-proj halves straddle the PV so big512 pool
                        # rotation (pO copy latency) never head-of-line
                        # blocks the PE queue; late frames avoid DVE copies
                        # (DVE runs recip+normalize there)
                        pat = c["late_pat"] if f >= 6 else None
                        if c["pvn_first"]:
                            # pv_norm emitted before scores(f): GPSIMD's
                            # in-order queue then runs the norm broadcasts
                            # before frame f's (slow) Pool mask multiplies
                            if f >= 1:
                                pv_norm(f - 1)
                            if f >= 2:
                                o_half(f - 2, 0, pat=pat)
                            scores_emk(f)
                            if f >= 2:
                                o_half(f - 2, 1, pat=pat)
                        else:
                            scores_emk(f)
                            if f >= 2:
                                o_half(f - 2, 0, pat=pat)
                            if f >= 1:
                                pv_norm(f - 1)
                            if f >= 2:
                                o_half(f - 2, 1, pat=pat)
                        if f == F - 1:
                            o_half(F - 2, 0)
                            if c["o6_pull"]:
                                o_half(F - 2, 1)
                    pv_norm(F - 1, split=True)
                    if not c["o6_pull"]:
                        o_half(F - 2, 1)
                    o_half(F - 1, 0, pat="ad", tail=True)
                    o_half(F - 1, 1, pat="da", tail=True)

            if debug:
                nc.sync.dma_start(dbg["qT"][:], qT_sb[:])
                nc.sync.dma_start(dbg["kT"][:], kT_sb[:])
                nc.sync.dma_start(dbg["v1"][:], v1_sb[:])
                nc.sync.dma_start(dbg["rp"][:], rp_sb[:])
                nc.sync.dma_start(dbg["oTn"][:], oTn_sb[:])

    nc.compile()
    _nc_cache[key] = nc
    return nc


def _fp8_hilo(a32, lo_scale):
    """fp8 hi + fp8 of the scaled residual."""
    hi = a32.astype(NP8)
    lo = ((a32 - hi.astype(np.float32)) * lo_scale).astype(NP8)
    return hi, lo


def make_in_maps(hidden_states, Wq, Wk, Wv, Wo):
    """Host-side shard + repack of full inputs into per-core input maps."""
    hid = np.asarray(hidden_states, np.float32).reshape(S, D)
    # hidT packed [ki, ko, s] with d = ko*128 + ki
    hidT_pk = np.ascontiguousarray(hid.T.reshape(KC, P, S).transpose(1, 0, 2))
    xh8, xl8 = _fp8_hilo(hidT_pk, 32.0)

    Wq_ = np.asarray(Wq, np.float32)
    Wk_ = np.asarray(Wk, np.float32)
    Wv_ = np.asarray(Wv, np.float32)
    Wo_ = np.asarray(Wo, np.float32)

    def pack_w3(W, cidx):
        Wc = W[:, cidx * HPC * HD : (cidx + 1) * HPC * HD]  # [D, 128]
        wpk = np.ascontiguousarray(
            Wc.reshape(KC, P, HPC * HD).transpose(1, 0, 2)
        )
        hi = (wpk * 256.0).astype(NP8)
        lo = ((wpk * 256.0 - hi.astype(np.float32))).astype(NP8)
        wp = (wpk * 8.0).astype(NP8)
        return np.ascontiguousarray(np.stack([hi, wp, lo], axis=1))

    # (h, w) window mask, 0/1, [256, 256] (symmetric), packed to the live
    # window layout [p, j, c]: m01_pk[p, j, c] = W01[j*128 + p, 96*j + c]
    idx = np.arange(T)
    hh, ww = idx // GW, idx % GW
    m = (np.abs(hh[:, None] - hh[None, :]) <= WH // 2) & (
        np.abs(ww[:, None] - ww[None, :]) <= WW // 2
    )
    m01_pk = np.empty((P, 4, LQ), np.float16)
    for j in range(4):
        jp = j % 2
        m01_pk[:, j, :] = m[jp * P : (jp + 1) * P, 96 * jp : 96 * jp + LQ]

    in_maps = []
    for cidx in range(NCORES):
        wq3 = pack_w3(Wq_, cidx)
        in_maps.append(
            dict(
                xh=xh8,
                xl=xl8,
                wqh=np.ascontiguousarray(wq3[:, 0]),
                wqpl=np.ascontiguousarray(wq3[:, 1:]),
                wk3=pack_w3(Wk_, cidx),
                wv3=pack_w3(Wv_, cidx),
                wo=(
                    Wo_[cidx * HPC * HD : (cidx + 1) * HPC * HD, :] / 256.0
                ).astype(np.float16),
                m01=m01_pk,
            )
        )
    return in_maps


def kernel(
    hidden_states,
    Wq,
    Wk,
    Wv,
    Wo,
    bo,
    frames=F,
    height=GH,
    width=GW,
    wf=WF,
    wh=WH,
    ww=WW,
):
    assert (int(frames), int(height), int(width)) == (F, GH, GW)
    assert (int(wf), int(wh), int(ww)) == (WF, WH, WW)
    in_maps = make_in_maps(hidden_states, Wq, Wk, Wv, Wo)
    nc = build_nc(debug=False)
    res = run_bass_kernel_spmd(nc, in_maps, core_ids=list(range(NCORES)))
    acc = np.zeros((S, D), np.float32)
    for r in res.results:
        acc += r["out"].reshape(S, D)
    acc += np.asarray(bo, np.float32)[None, :]
    return acc.reshape(1, S, D)


# revision 63
# speedup vs baseline: 1.0448x; 1.0376x over previous
"""Trainium2 Bass kernel: 3D-window sparse multi-head attention.

Full op: out = SDPA(hid@Wq, hid@Wk, hid@Wv; 3D local window mask) @ Wo + bo
Shapes: hid [1, 2048, 1024], 16 heads x 64, grid (8 frames, 16, 16), window (3, 5, 5).

Sharding: head-parallel. Each of the 8 cores computes 2 heads end-to-end
(QKV projection slices, windowed attention, Wo row-slice projection) and
writes a full-shape fp16 partial; the host sums the 8 partials and adds bo.

v2 over the fp16 baseline (61.1us -> 54.0us TimelineSim):
  * q/k/v projections run in fp8 (e4m3) DoubleRow perf mode: one DR matmul
    contracts TWO 128-row k-tiles at 0.5 cycles/col, so a D=1024 contraction
    takes 4 DRs instead of 8 fp16 matmuls.  Precision is recovered with a
    hi+lo split of BOTH operands (x = xh + xl/32, W = Wh/256 + Wl/256):
    three cross terms (xh*Wh, xl*Wp, xh*Wl) at product scale 256 accumulate
    in one PSUM group -> 12 DRs = 0.75x the fp16 PE cost, ~1.6e-3 rel err
    measured.  All scale factors are powers of 2 folded into host packing,
    the exp's constant scale (2^-19 = 1/(256*256*sqrt(hd))), and host-side
    Wo/256 (the v path carries a 256x scale through v1/oTn; fp16 range is
    fine: oTn sigma ~80, max ~1400).
  * software pipeline per iteration f: projection chunks land just before
    the first frame needing them, then scores(f) | o-half(f-2) | PV+norm
    (f-1) | o-half(f-2); the o-proj halves straddle the PV so pO copy
    latency never head-of-line blocks the in-order PE queue.
  * engine placement (hw rule: GPSIMD cannot touch PSUM): exp + q/k copies
    on ACT, masks + reciprocal + normalize + v copy on DVE, o-proj copies
    cycle ACT/DVE/DVE/ACT, broadcasts on GPSIMD.  Weight trios + m01 ride
    the SWDGE queue (descriptor gen on the idle Pool engine) and x blocks
    2-3 load as one coalesced DMA each, because HWDGE descriptor
    generation is a single serial ~625ns/DMA resource.
  * PSUM banks (8): big512 pool (3 bufs) shared by q/k-proj and o-proj,
    paired two-head v tiles, psS x2, two-head pso [66, 2, 256] x2 (one
    [1,512] reciprocal per frame covers both heads' denominators).
  * tail: frame 7 normalizes per head and per seq-chunk half so the last
    o-proj starts early; the frame-6 o-proj copies run ACT-only so the DVE
    queue stays clear for the final recip+normalize chain; xl block 0 rides
    SWDGE to keep the early HWDGE gen queue short.
  * NOTE hw pitfalls (verified on device): GPSIMD+PSUM fails BIR
    verification; per-head K=64 o-proj accumulation (tail_split) dies at
    runtime; both are kept off.
"""

import numpy as np
import ml_dtypes

import concourse.bass as bass
import concourse.mybir as mybir
import concourse.tile as tile
from concourse import bacc
from concourse.bass import ds, ts
from concourse.bass_utils import run_bass_kernel_spmd

S, D, NH, HD = 2048, 1024, 16, 64
NCORES = 8
HPC = NH // NCORES          # heads per core = 2
F, GH, GW = 8, 16, 16       # frames, height, width (S = F*GH*GW)
WF, WH, WW = 3, 5, 5        # window sizes
T = GH * GW                 # tokens per frame = 256
P = 128
KC = D // P                 # 8 contraction chunks
SC = S // P                 # 16 seq chunks of 128
NQ = S // 512               # 4 free chunks of 512
LQ = 160                    # live q columns per kv chunk (h-band of the mask)
F16 = mybir.dt.float16
F32 = mybir.dt.float32
F8 = mybir.dt.float8e4
EXP = mybir.ActivationFunctionType.Exp
MUL = mybir.AluOpType.mult
DR = mybir.MatmulPerfMode.DoubleRow
NP8 = ml_dtypes.float8_e4m3fn
EXP_SCALE = float(2.0 ** -19)   # undo 256*256 packing and apply 1/sqrt(hd)

_nc_cache = {}

# engine-assignment / buffering knobs.
# NOTE: GPSIMD ("p") cannot touch PSUM on hardware — PSUM->SBUF copies and
# the normalize TT must stay on ACT ("a") / DVE ("d"); Pool is only usable
# for SBUF-only work (partition broadcast, the mask multiplies).
CFG = dict(
    et_bufs=6, pm_bufs=8, ob_bufs=8, pre_v=1, late_pat="daa", tail_swdge=False, pvn_first=False, o6_pull=True, warmup=0, tail_halves=False, s7_early=False, s7_meng="d", split_fine=False, tail_p0="ad", tail_p1="da", qk0_cs=False, dma_coal=True, o_coal=False, xl0_gp=True, wqpl_gp=False, xl1_gp=False,
    eng_qk="a",        # psq/psk -> qT/kT copies: a=ACT, d=DVE
    eng_v="d",         # psv -> v1 copies
    eng_norm="d",      # normalize TT
    o_pat="adda",      # o-proj copy engines, cycling per copy
    mask_pat="d",      # mask-multiply engines, cycling per op
    dma_w_gp=True,     # wv3 on the gpsimd (SWDGE) queue
    dma_hid_vec=False, # stripe hid blocks across sync+scalar queues
    dma_out_sc=False,  # out DMAs on the scalar queue instead of sync
    tail_split=False,  # per-head K=64 o-proj: BROKEN on hw (runtime error)
)

_ENG = {"a": "scalar", "d": "vector", "p": "gpsimd"}


def _copy(nc, eng, dst, src):
    e = getattr(nc, _ENG[eng])
    if eng == "a":
        e.copy(dst, src)
    else:
        e.tensor_copy(dst, src)


def build_nc(debug=False, repeat=1, **cfg):
    c = dict(CFG, **cfg)
    key = (bool(debug), repeat, tuple(sorted(c.items())))
    if key in _nc_cache:
        return _nc_cache[key]
    nc = bacc.Bacc(None, target_bir_lowering=False, debug=False)

    xh = nc.dram_tensor("xh", [P, KC, S], F8, kind="ExternalInput")
    xl = nc.dram_tensor("xl", [P, KC, S], F8, kind="ExternalInput")
    # per-GEMM weight trios, term-major: [hi(256W), p(8W), lo(256*resid)];
    # wq is split so the hi term can land before the full trio streams in
    wqh = nc.dram_tensor("wqh", [P, KC, P], F8, kind="ExternalInput")
    wqpl = nc.dram_tensor("wqpl", [P, 2, KC, P], F8, kind="ExternalInput")
    wk3 = nc.dram_tensor("wk3", [P, 3, KC, P], F8, kind="ExternalInput")
    wv3 = nc.dram_tensor("wv3", [P, 3, KC, P], F8, kind="ExternalInput")
    wo = nc.dram_tensor("wo", [P, D], F16, kind="ExternalInput")
    m01 = nc.dram_tensor("m01", [P, 4, LQ], F16, kind="ExternalInput")
    out = nc.dram_tensor("out", [SC, P, D], F16, kind="ExternalOutput")
    dbg = {}
    if debug:
        dbg["qT"] = nc.dram_tensor("dbg_qt", [P, S], F16, kind="ExternalOutput")
        dbg["kT"] = nc.dram_tensor("dbg_kt", [P, S], F16, kind="ExternalOutput")
        dbg["v1"] = nc.dram_tensor("dbg_v1", [P, SC, HPC, 66], F16, kind="ExternalOutput")
        dbg["rp"] = nc.dram_tensor("dbg_rp", [1, 2 * S], F16, kind="ExternalOutput")
        dbg["oTn"] = nc.dram_tensor("dbg_otn", [P, S], F16, kind="ExternalOutput")

    with tile.TileContext(nc) as tc:
        with (
            tc.tile_pool(name="const", bufs=1) as cpool,
            tc.tile_pool(name="qk", bufs=1) as qkpool,
            tc.tile_pool(name="attn", bufs=4) as apool,
            tc.tile_pool(name="acc", bufs=1) as accpool,
            tc.tile_pool(name="ostage", bufs=4) as opool,
        ):
            # ---- constant loads. sync: wq (hi first) + xh block 0 pairs;
            # scalar: remaining xh-b0 pairs + all xl blocks; SWDGE: wk3, wv3,
            # m01 (Pool is idle early; each costs ~1us of descriptor gen).
            wqh_sb = cpool.tile([P, KC, P], F8, tag="wqh")
            wqpl_sb = cpool.tile([P, 2, KC, P], F8, tag="wqpl")
            wk3_sb = cpool.tile([P, 3, KC, P], F8, tag="wk3")
            wv3_sb = cpool.tile([P, 3, KC, P], F8, tag="wv3")
            xh_sb = cpool.tile([P, KC, S], F8, tag="xh")
            xl_sb = cpool.tile([P, KC, S], F8, tag="xl")
            m01_sb = cpool.tile([P, 4, LQ], F16, tag="m01")
            wo_sb = cpool.tile([P, D], F16, tag="wo")

            def xh_pair(eng, tp, nch=0):
                eng.dma_start(
                    xh_sb[:, 2 * tp : 2 * tp + 2, ts(nch, 512)],
                    xh[:, 2 * tp : 2 * tp + 2, ts(nch, 512)],
                )

            heng = nc.scalar if c["dma_hid_vec"] else nc.sync
            weng = nc.gpsimd if c["dma_w_gp"] else nc.sync
            nc.sync.dma_start(wqh_sb[:], wqh[:])
            if c["qk0_cs"]:
                # block 0 split by columns: frame 0's scores need only
                # q cols 0:256 and k cols 0:384, so cols 0:384 of xh/xl
                # unblock the whole first scores group
                nc.sync.dma_start(xh_sb[:, :, 0:384], xh[:, :, 0:384])
                weng.dma_start(wk3_sb[:], wk3[:])
                (weng if c["wqpl_gp"] else nc.sync).dma_start(wqpl_sb[:], wqpl[:])
                heng.dma_start(xl_sb[:, :, 0:384], xl[:, :, 0:384])
                nc.sync.dma_start(xh_sb[:, :, 384:512], xh[:, :, 384:512])
                heng.dma_start(xl_sb[:, :, 384:512], xl[:, :, 384:512])
            else:
                xh_pair(heng, 0)
                weng.dma_start(wk3_sb[:], wk3[:])
                (weng if c["wqpl_gp"] else nc.sync).dma_start(wqpl_sb[:], wqpl[:])
                xh_pair(heng, 2)
                xh_pair(nc.sync, 1)
                xl0eng = weng if c["xl0_gp"] else heng
                xl0eng.dma_start(xl_sb[:, 0:4, ts(0, 512)], xl[:, 0:4, ts(0, 512)])
                xh_pair(nc.sync, 3)
                (weng if c["xl0_gp"] else nc.sync).dma_start(
                    xl_sb[:, 4:8, ts(0, 512)], xl[:, 4:8, ts(0, 512)])
            weng.dma_start(wv3_sb[:], wv3[:])
            weng.dma_start(m01_sb[:], m01[:])
            nc.sync.dma_start(xh_sb[:, :, ts(1, 512)], xh[:, :, ts(1, 512)])
            (weng if c["xl1_gp"] else heng).dma_start(
                xl_sb[:, :, ts(1, 512)], xl[:, :, ts(1, 512)])
            nc.sync.dma_start(wo_sb[:], wo[:])
            if c["dma_coal"]:
                nc.sync.dma_start(xh_sb[:, :, 1024:2048], xh[:, :, 1024:2048])
                heng.dma_start(xl_sb[:, :, 1024:2048], xl[:, :, 1024:2048])
            else:
                for nch in range(2, NQ):
                    nc.sync.dma_start(xh_sb[:, :, ts(nch, 512)], xh[:, :, ts(nch, 512)])
                    heng.dma_start(xl_sb[:, :, ts(nch, 512)], xl[:, :, ts(nch, 512)])

            qT_sb = qkpool.tile([P, S], F16, tag="qT")
            kT_sb = qkpool.tile([P, S], F16, tag="kT")
            # sc-major v with an appended ones column for the denominator;
            # [P, sc, h, 66] lets one strided copy stage both heads of a
            # psv pair at once
            v1_sb = qkpool.tile([P, SC, HPC, 66], F16, tag="v1")
            oTn_sb = accpool.tile([P, S], F16, tag="oTn")
            # reciprocal denominators, frame-major [f*512 + h*256 + t]
            rp_sb = accpool.tile([1, HPC * S], F16, tag="rp")

            nc.vector.memset(v1_sb[:, :, 0, HD : HD + 2], 1.0)
            nc.vector.memset(v1_sb[:, :, 1, HD : HD + 2], 1.0)

            with (
                tc.tile_pool(name="big512", bufs=3, space="PSUM") as big512,
                tc.tile_pool(name="pv", bufs=1, space="PSUM") as pvp,
                tc.tile_pool(name="psS", bufs=2, space="PSUM") as pssp,
                tc.tile_pool(name="pso", bufs=2, space="PSUM") as psop,
                nc.allow_low_precision("softmax reciprocal in fp16"),
            ):
                o_ei = [0]  # cycling index into o_pat
                m_ei = [0]  # cycling index into mask_pat

                if c["warmup"]:
                    # dummy matmuls on a junk tile: the PE p-state ramps to
                    # full speed after ~3us of continuous execution, and the
                    # DMA-gated start would otherwise keep resetting it, so
                    # the first real DRs would run at half speed
                    junk = qkpool.tile([P, 512], F16, tag="junk")
                    nc.vector.memset(junk[:, 0:P], 0.0)
                    pw = pssp.tile([P, 3, LQ], F32, tag="psS", name="warm")
                    for i in range(c["warmup"]):
                        nc.tensor.matmul(
                            pw[:], junk[:, 0:P], junk[:, 0:480],
                            start=(i == 0), stop=(i == c["warmup"] - 1),
                        )

                def qw(term, t):
                    if term == 0:
                        return wqh_sb[:, 2 * t : 2 * t + 2, :]
                    return wqpl_sb[:, term - 1, 2 * t : 2 * t + 2, :]

                def kw(term, t):
                    return wk3_sb[:, term, 2 * t : 2 * t + 2, :]

                # term emission order: hi (xh), lo-W (xh), p (xl) — terms
                # needing xl go last so xl DMAs can trail xh
                TERMS = ((0, 0), (2, 0), (1, 1))

                def proj_drs(ps, wf, nch, terms=TERMS, n0=0, ntot=12):
                    """DoubleRow matmuls accumulating 256*(W^T x) for one
                    512-column chunk."""
                    n = n0
                    xs = (xh_sb, xl_sb)
                    for term, xi in terms:
                        for t in range(KC // 2):
                            nc.tensor.matmul(
                                ps[:], wf(term, t),
                                xs[xi][:, 2 * t : 2 * t + 2, ts(nch, 512)],
                                start=(n == 0), stop=(n == ntot - 1),
                                perf_mode=DR,
                            )
                            n += 1
                    return n

                def q_chunk(nch):
                    psq = big512.tile([P, 512], F32, tag="big", name="psq")
                    proj_drs(psq, qw, nch)
                    _copy(nc, c["eng_qk"], qT_sb[:, ts(nch, 512)], psq[:])

                def k_chunk(nch):
                    psk = big512.tile([P, 512], F32, tag="big", name="psk")
                    proj_drs(psk, kw, nch)
                    _copy(nc, c["eng_qk"], kT_sb[:, ts(nch, 512)], psk[:])

                def proj_drs_cols(ps, wf, c0, c1, xoff=0):
                    """12 DR accumulation over x columns [c0:c1)."""
                    n = 0
                    xs = (xh_sb, xl_sb)
                    for term, xi in TERMS:
                        for t in range(KC // 2):
                            nc.tensor.matmul(
                                ps[:, c0 - xoff : c1 - xoff], wf(term, t),
                                xs[xi][:, 2 * t : 2 * t + 2, c0:c1],
                                start=(n == 0), stop=(n == 11),
                                perf_mode=DR,
                            )
                            n += 1

                def qk_chunk(nch, interleave=False):
                    if interleave and c["qk0_cs"]:
                        # column-split: the A parts (q 0:256, k 0:384) are
                        # all frame 0's scores need; B parts follow once the
                        # 384:512 slices land
                        psq = big512.tile([P, 512], F32, tag="big", name="psq")
                        psk = big512.tile([P, 512], F32, tag="big", name="psk")
                        proj_drs_cols(psq, qw, 0, 256)
                        proj_drs_cols(psk, kw, 0, 384)
                        _copy(nc, c["eng_qk"], qT_sb[:, 0:256], psq[:, 0:256])
                        _copy(nc, c["eng_qk"], kT_sb[:, 0:384], psk[:, 0:384])
                        proj_drs_cols(psk, kw, 384, 512)
                        proj_drs_cols(psq, qw, 256, 512)
                        _copy(nc, c["eng_qk"], kT_sb[:, 384:512], psk[:, 384:512])
                        _copy(nc, c["eng_qk"], qT_sb[:, 256:512], psq[:, 256:512])
                    elif interleave:
                        # ordered by DMA arrival: wqh, wk3, wqpl, xl halves
                        psq = big512.tile([P, 512], F32, tag="big", name="psq")
                        psk = big512.tile([P, 512], F32, tag="big", name="psk")
                        for ti in range(3):
                            proj_drs(psq, qw, nch, terms=TERMS[ti : ti + 1],
                                     n0=4 * ti)
                            proj_drs(psk, kw, nch, terms=TERMS[ti : ti + 1],
                                     n0=4 * ti)
                        _copy(nc, c["eng_qk"], qT_sb[:, ts(nch, 512)], psq[:])
                        _copy(nc, c["eng_qk"], kT_sb[:, ts(nch, 512)], psk[:])
                    else:
                        q_chunk(nch)
                        k_chunk(nch)

                def v_pair(pair):
                    """v projection for seq chunks 2*pair, 2*pair+1."""
                    psv = pvp.tile([P, 2, HPC, HD], F32, tag="psv")
                    for j in range(2):
                        sc = 2 * pair + j
                        n = 0
                        for term, x_sb in ((0, xh_sb), (2, xh_sb), (1, xl_sb)):
                            for t in range(KC // 2):
                                nc.tensor.matmul(
                                    psv[:, j],
                                    x_sb[:, 2 * t : 2 * t + 2, ts(sc, P)],
                                    wv3_sb[:, term, 2 * t : 2 * t + 2, :],
                                    start=(n == 0), stop=(n == 11),
                                    perf_mode=DR,
                                )
                                n += 1
                    _copy(
                        nc, c["eng_v"],
                        v1_sb[:, 2 * pair : 2 * pair + 2, :, 0:HD],
                        psv[:],
                    )

                obf = {}

                def o_half(f, j, pat=None, tail=False):
                    """output projection for seq chunk 2f+j."""
                    split = tail and c["tail_split"]
                    heads = (0, 1) if split else (None,)
                    sc = 2 * f + j
                    if c["o_coal"] and not tail:
                        if j == 0:
                            obf[f] = opool.tile(
                                [P, 2, D], F16, tag="ob2",
                                bufs=c["ob_bufs"] // 2, name="ob2t",
                            )
                        ob = obf[f][:, j]
                    else:
                        ob = opool.tile([P, D], F16, tag="ob", bufs=c["ob_bufs"])
                    oeng = nc.scalar if c["dma_out_sc"] else nc.sync
                    for n2 in range(2):
                        pO = big512.tile([P, 512], F32, tag="big", name="pO")
                        for i, h in enumerate(heads):
                            hr = ds(0, P) if h is None else ds(h * HD, HD)
                            nc.tensor.matmul(
                                pO[:], oTn_sb[hr, ts(sc, P)],
                                wo_sb[hr, ts(n2, 512)],
                                start=(i == 0), stop=(i == len(heads) - 1),
                            )
                        pp = pat or c["o_pat"]
                        e = pp[o_ei[0] % len(pp)]
                        o_ei[0] += 1
                        _copy(nc, e, ob[:, ts(n2, 512)], pO[:])
                        if tail and c["tail_halves"]:
                            # per-half DMAs overlap copy completion with the
                            # serial HWDGE descriptor-gen pipeline
                            te = nc.gpsimd if (j == 1 and c["tail_swdge"]) else oeng
                            te.dma_start(out[sc, :, ts(n2, 512)], ob[:, ts(n2, 512)])
                    if tail and not c["tail_halves"]:
                        te = nc.gpsimd if (j == 1 and c["tail_swdge"]) else oeng
                        te.dma_start(out[sc, :, :], ob[:])
                    if not tail:
                        if c["o_coal"]:
                            if j == 1:
                                oeng.dma_start(
                                    out[2 * f : 2 * f + 2, :, :], obf.pop(f)[:]
                                )
                        else:
                            oeng.dma_start(out[sc, :, :], ob[:])

                def fgroups(f):
                    lo, hi = max(0, f - 1), min(F - 1, f + 1)
                    chunks = list(range(2 * lo, 2 * hi + 2))
                    return [chunks[i : i + 3] for i in range(0, len(chunks), 3)]

                pms = {}

                def scores_emk(f, meng_ovr=None):
                    """scores + exp + mask for both heads of frame f.
                    The (h,w) window implies a kv-h band: an even kv chunk
                    (kv h 0..7) only reaches q columns 0:160, an odd chunk
                    (kv h 8..15) only 96:256. Tiles hold just those 160
                    live columns (m01 is host-packed the same way); PSUM's
                    per-element has_written bits make the partial-coverage
                    PV accumulation exact."""
                    for h in range(HPC):
                        hr = ds(h * HD, HD)
                        for b, g in enumerate(fgroups(f)):
                            n = len(g)
                            psS = pssp.tile([P, 3, LQ], F32, tag="psS")
                            for i, ck in enumerate(g):
                                nc.tensor.matmul(
                                    psS[:, i, :],
                                    kT_sb[hr, ds(ck * P, P)],
                                    qT_sb[hr, ds(f * T + 96 * (ck % 2), LQ)],
                                    start=True, stop=True,
                                )
                            et = apool.tile(
                                [P, 3, LQ], F16, tag="et", bufs=c["et_bufs"]
                            )
                            nc.scalar.activation(
                                et[:, 0:n, :], psS[:, 0:n, :], EXP, scale=EXP_SCALE
                            )
                            pm = apool.tile(
                                [P, 3, LQ], F16, tag="pm", bufs=c["pm_bufs"]
                            )
                            p0 = g[0] % 2
                            me = meng_ovr or c["mask_pat"][m_ei[0] % len(c["mask_pat"])]
                            m_ei[0] += 1
                            meng = nc.gpsimd if me == "p" else nc.vector
                            meng.tensor_tensor(
                                pm[:, 0:n, :], et[:, 0:n, :],
                                m01_sb[:, p0 : p0 + n, :], MUL,
                            )
                            pms[(f, h, b)] = pm

                def pv_norm(f, split=False):
                    """PV for both heads into one two-head PSUM tile, then
                    one reciprocal over both denominator rows + per-head
                    GPSIMD broadcast + normalize multiply. With split=True
                    the normalize runs per seq-chunk half so the tail
                    o-projection can start off the first half early."""
                    fs = ds(f * T, T)
                    groups = fgroups(f)
                    pso = psop.tile([66, HPC, T], F32, tag="pso")
                    for h in range(HPC):
                        for b, g in enumerate(groups):
                            pm = pms.pop((f, h, b))
                            for i, ck in enumerate(g):
                                nc.tensor.matmul(
                                    pso[0 : HD + 1, h, ds(96 * (ck % 2), LQ)],
                                    v1_sb[:, ck, h, 0 : HD + 1],
                                    pm[:, i, :],
                                    start=(b == 0 and i == 0),
                                    stop=(b == len(groups) - 1 and i == len(g) - 1),
                                )
                    e = nc.vector if c["eng_norm"] == "d" else nc.gpsimd
                    if not split:
                        rps = ds(f * HPC * T, HPC * T)
                        nc.vector.reciprocal(rp_sb[0:1, rps], pso[HD : HD + 1, :, :])
                        for h in range(HPC):
                            rph = ds(f * HPC * T + h * T, T)
                            pbs = apool.tile([HD, T], F16, tag="pbs", bufs=2)
                            nc.gpsimd.partition_broadcast(pbs[:], rp_sb[0:1, rph])
                            e.tensor_tensor(
                                oTn_sb[ds(h * HD, HD), fs], pso[0:HD, h, :], pbs[:], MUL
                            )
                    elif c["split_fine"]:
                        # tail variant: per-(head, seq-chunk-half) recip and
                        # broadcast so the final o-projection's first chunk
                        # unblocks as early as possible
                        pbss = {}
                        for half in range(2):
                            for h in range(HPC):
                                rph = ds(f * HPC * T + h * T + half * P, P)
                                nc.vector.reciprocal(
                                    rp_sb[0:1, rph],
                                    pso[HD : HD + 1, h, ds(half * P, P)],
                                )
                                pbs = apool.tile([HD, P], F16, tag="pbf", bufs=4)
                                nc.gpsimd.partition_broadcast(pbs[:], rp_sb[0:1, rph])
                                pbss[(h, half)] = pbs
                        for half in range(2):
                            for h in range(HPC):
                                e.tensor_tensor(
                                    oTn_sb[ds(h * HD, HD), ds(f * T + half * P, P)],
                                    pso[0:HD, h, ds(half * P, P)],
                                    pbss[(h, half)][:], MUL,
                                )
                    else:
                        pbss = {}
                        for h in range(HPC):
                            rph = ds(f * HPC * T + h * T, T)
                            nc.vector.reciprocal(
                                rp_sb[0:1, rph], pso[HD : HD + 1, h, :]
                            )
                            pbs = apool.tile([HD, T], F16, tag="pbs", bufs=2)
                            nc.gpsimd.partition_broadcast(pbs[:], rp_sb[0:1, rph])
                            pbss[h] = pbs
                        for half in range(2):
                            hs = ds(half * P, P)
                            for h in range(HPC):
                                e.tensor_tensor(
                                    oTn_sb[ds(h * HD, HD), ds(f * T + half * P, P)],
                                    pso[0:HD, h, hs], pbss[h][:, hs], MUL,
                                )

                # Software pipeline, per iteration f: projection chunks land
                # just before the first frame that needs them (vp_k by
                # iteration k, kT chunk n by the first frame whose kv band
                # reaches it, qT chunk n by frame 2n); scores(f) issue while
                # frame f-1's PV and frame f-2's output projection fill PE,
                # hiding the exp -> mask chain and the normalize chain
                # (recip -> broadcast -> multiply). Chunks are spread so
                # every iteration has PE filler.
                pres = {
                    1: {0: ["qk0", "vp0", "vp1"], 1: ["qk1", "vp2"],
                        2: ["vp3"], 3: ["qk2", "vp4"], 4: ["vp5"],
                        5: ["qk3", "vp6"], 6: ["vp7"], 7: []},
                    2: {0: ["qk0", "vp0"], 1: ["qk1", "vp1"],
                        2: ["vp2", "vp3"], 3: ["qk2", "vp4"],
                        4: ["vp5", "vp6"], 5: ["k3"], 6: ["q3", "vp7"],
                        7: []},
                    3: {0: ["qk0", "vp0"], 1: ["qk1", "vp1"],
                        2: ["vp2", "vp3"], 3: ["qk2", "vp4"],
                        4: ["vp5"], 5: ["k3", "vp6"], 6: ["q3", "vp7"],
                        7: []},
                    4: {0: ["qk0", "vp0", "vp1"], 1: ["qk1", "vp2"],
                        2: ["vp3", "vp4"], 3: ["qk2", "vp5"],
                        4: ["vp6"], 5: ["qk3", "vp7"], 6: [], 7: []},
                    5: {0: ["qk0", "vp0", "vp1"], 1: ["qk1", "vp2"],
                        2: ["vp3"], 3: ["qk2", "vp4"], 4: ["vp5"],
                        5: ["qk3", "vp6"], 6: [], 7: ["vp7"]},
                }
                pre = pres[c["pre_v"]]
                for rep in range(repeat):
                    for f in range(F):
                        for w in pre[f]:
                            if w.startswith("qk"):
                                nch = int(w[2:])
                                qk_chunk(nch, interleave=(nch == 0 and rep == 0))
                            elif w.startswith("q"):
                                q_chunk(int(w[1:]))
                            elif w.startswith("k"):
                                k_chunk(int(w[1:]))
                            else:
                                v_pair(int(w[2:]))
                        # o-proj halves straddle the PV so big512 pool
                        # rotation (pO copy latency) never head-of-line
                        # blocks the PE queue; late frames avoid DVE copies
                        # (DVE runs recip+normalize there)
                        pat = c["late_pat"] if f >= 6 else None
                        if c["pvn_first"]:
                            # pv_norm emitted before scores(f): GPSIMD's
                            # in-order queue then runs the norm broadcasts
                            # before frame f's (slow) Pool mask multiplies
                            if f >= 1:
                                pv_norm(f - 1)
                            if f >= 2:
                                o_half(f - 2, 0, pat=pat)
                            scores_emk(f)
                            if f >= 2:
                                o_half(f - 2, 1, pat=pat)
                        else:
                            scores_emk(f)
                            if f >= 2:
                                o_half(f - 2, 0, pat=pat)
                            if f >= 1:
                                pv_norm(f - 1)
                            if f >= 2:
                                o_half(f - 2, 1, pat=pat)
                        if f == F - 2 and c["s7_early"]:
                            scores_emk(F - 1, meng_ovr=c["s7_meng"])
                        if f == F - 1:
                            # ACT-only copies: keep the DVE queue clear for
                            # the final frame's recip + normalize chain
                            o_half(F - 2, 0, pat="a")
                            if c["o6_pull"]:
                                o_half(F - 2, 1, pat="a")
                    pv_norm(F - 1, split=True)
                    if not c["o6_pull"]:
                        o_half(F - 2, 1)
                    o_half(F - 1, 0, pat=c["tail_p0"], tail=True)
                    o_half(F - 1, 1, pat=c["tail_p1"], tail=True)

            if debug:
                nc.sync.dma_start(dbg["qT"][:], qT_sb[:])
                nc.sync.dma_start(dbg["kT"][:], kT_sb[:])
                nc.sync.dma_start(dbg["v1"][:], v1_sb[:])
                nc.sync.dma_start(dbg["rp"][:], rp_sb[:])
                nc.sync.dma_start(dbg["oTn"][:], oTn_sb[:])

    nc.compile()
    _nc_cache[key] = nc
    return nc


def _fp8_hilo(a32, lo_scale):
    """fp8 hi + fp8 of the scaled residual."""
    hi = a32.astype(NP8)
    lo = ((a32 - hi.astype(np.float32)) * lo_scale).astype(NP8)
    return hi, lo


def make_in_maps(hidden_states, Wq, Wk, Wv, Wo):
    """Host-side shard + repack of full inputs into per-core input maps."""
    hid = np.asarray(hidden_states, np.float32).reshape(S, D)
    # hidT packed [ki, ko, s] with d = ko*128 + ki
    hidT_pk = np.ascontiguousarray(hid.T.reshape(KC, P, S).transpose(1, 0, 2))
    xh8, xl8 = _fp8_hilo(hidT_pk, 32.0)

    Wq_ = np.asarray(Wq, np.float32)
    Wk_ = np.asarray(Wk, np.float32)
    Wv_ = np.asarray(Wv, np.float32)
    Wo_ = np.asarray(Wo, np.float32)

    def pack_w3(W, cidx):
        Wc = W[:, cidx * HPC * HD : (cidx + 1) * HPC * HD]  # [D, 128]
        wpk = np.ascontiguousarray(
            Wc.reshape(KC, P, HPC * HD).transpose(1, 0, 2)
        )
        hi = (wpk * 256.0).astype(NP8)
        lo = ((wpk * 256.0 - hi.astype(np.float32))).astype(NP8)
        wp = (wpk * 8.0).astype(NP8)
        return np.ascontiguousarray(np.stack([hi, wp, lo], axis=1))

    # (h, w) window mask, 0/1, [256, 256] (symmetric), packed to the live
    # window layout [p, j, c]: m01_pk[p, j, c] = W01[j*128 + p, 96*j + c]
    idx = np.arange(T)
    hh, ww = idx // GW, idx % GW
    m = (np.abs(hh[:, None] - hh[None, :]) <= WH // 2) & (
        np.abs(ww[:, None] - ww[None, :]) <= WW // 2
    )
    m01_pk = np.empty((P, 4, LQ), np.float16)
    for j in range(4):
        jp = j % 2
        m01_pk[:, j, :] = m[jp * P : (jp + 1) * P, 96 * jp : 96 * jp + LQ]

    in_maps = []
    for cidx in range(NCORES):
        wq3 = pack_w3(Wq_, cidx)
        in_maps.append(
            dict(
                xh=xh8,
                xl=xl8,
                wqh=np.ascontiguousarray(wq3[:, 0]),
                wqpl=np.ascontiguousarray(wq3[:, 1:]),
                wk3=pack_w3(Wk_, cidx),
                wv3=pack_w3(Wv_, cidx),
                wo=(
                    Wo_[cidx * HPC * HD : (cidx + 1) * HPC * HD, :] / 256.0
                ).astype(np.float16),
                m01=m01_pk,
            )
        )
    return in_maps


def kernel(
    hidden_states,
    Wq,
    Wk,
    Wv,
    Wo,
    bo,
    frames=F,
    height=GH,
    width=GW,
    wf=WF,
    wh=WH,
    ww=WW,
):
    assert (int(frames), int(height), int(width)) == (F, GH, GW)
    assert (int(wf), int(wh), int(ww)) == (WF, WH, WW)
    in_maps = make_in_maps(hidden_states, Wq, Wk, Wv, Wo)
    nc = build_nc(debug=False)
    res = run_bass_kernel_spmd(nc, in_maps, core_ids=list(range(NCORES)))
    acc = np.zeros((S, D), np.float32)
    for r in res.results:
        acc += r["out"].reshape(S, D)
    acc += np.asarray(bo, np.float32)[None, :]
    return acc.reshape(1, S, D)
